# revision 56
# baseline (speedup 1.0000x reference)
"""Trainium2 Bass kernel for DCMLayer: 1x1 conv -> per-sample dynamic 3x3
depthwise conv -> 1x1 fuse conv, data-parallel over 8 NeuronCores.

Contract: kernel(**inputs) takes the FULL unsharded inputs
(x[32,256,96,96], conv_w[64,256], conv_b[64], dw_b[64], fuse_w[256,64],
fuse_b[256]) and returns the full y[32,256,96,96] float32.

v5 (over v2's 202us baseline; ~176-179us fast-regime, the shared HW shows
~15% run-to-run throttling variance):
- x pre-cast to fp16 on host (the matmuls consume fp16 anyway; halves
  HBM reads) and host-tiled row-major so each half-strip load is one
  [128, 6144] DMA with 12.3KB contiguous descriptor lines (v2's 6KB
  lines were descriptor-rate-bound at ~60% HBM) and mm1 starts on the
  first half-strip.
- consts load FIRST on the sync ring: DMA queues round-robin
  descriptor-by-descriptor, so consts issued behind the strips trickle
  in ~30-50us late and gate the whole B phase.
- dw taps split PE (diagonal fp16 matmuls into PSUM) / Vector (4x-mode
  tensor_scalar_mul + 2x tensor_tensor add over 16-row groups),
  (5,4) mid-pack and (6,3) in the tail pack (Vector saturates there,
  PE has slack once mm1 is gone). GpSimd supports neither
  TensorScalarPtr nor PSUM access; fp8 dw blows the 2e-2 tolerance
  (measured 3e-2).
- per-chunk combine osb = pD + acc on Vector; mm2 as 512-col matmuls
  (PSUM bank max) over the group osb; samples a/b land in one 2-bank
  PSUM tile (b at col 512) -> single strided Scalar bias-evict per
  (mc, window); y stored per 16-row group ([128, 6144], 12.3KB lines),
  host untangles the layout.
- generator-woven software pipeline: pack-1 phase-A chunks interleave
  every other phase-B step (keeps pooling close behind mm1 so psA
  recycle never stalls); each group's mm2/evicts emit one group BEHIND
  its dw (PE's in-order queue never stalls on the psY/evict ping-pong);
  every tail group's back-phase starts mid-front (hybrid weave) and
  the last three groups' evicts alternate Scalar/Vector to share the
  end drain.
"""
import numpy as np

import concourse.bacc as bacc
import concourse.bass as bass
import concourse.tile as tile
from concourse import mybir
from concourse.bass_utils import run_bass_kernel_spmd

F32 = mybir.dt.float32
F16 = mybir.dt.float16
AF = mybir.ActivationFunctionType
ALU = mybir.AluOpType
AX = mybir.AxisListType

# Problem geometry (hardcoded per contract)
N, C, H, W = 32, 256, 96, 96
Cm, P = 64, 256
HW = H * W           # 9216
NCORES = 8
NLOC = N // NCORES   # 4 samples per core
NPACK = NLOC // 2    # 2 two-sample packs per core
KC4 = C // 64        # 4 K=64(x2 samples) contraction chunks for mm1
MC2 = P // 128       # 2 M=128 output chunks for mm2

WP = W + 2           # padded row width 98
FPAD = WP * (H + 2) + 2  # padded f buffer 9606 (+2 slack for corner taps)
RS = 4               # rows per compute chunk
NCH = H // RS        # 24 chunks per pack
NT = RS * W          # 384 = compute tile free size
NDW = RS * WP        # 392 dw output positions per chunk
GRV = 16             # rows per group (vector taps, osb, y store)
NTV = GRV * W        # 1536 = group free size
NQG = H // GRV       # 6 groups per pack
MMW = 512            # mm2 moving-window columns (PSUM bank max)
NW3 = NTV // MMW     # 3 mm2 windows per group
LR = 32              # rows per x load strip
NLD = H // LR        # 3 load strips per pack
BR = 32              # pooling block rows/cols

# dw tap split per (pack, qgroup): (PE diag-matmul taps, Vector taps).
# GpSimd/Pool supports neither TensorScalarPtr nor PSUM access, so taps go
# PE/DVE only. Alternating per group balances the two engines; the tail
# pack (no concurrent mm1) shifts one tap toward PE.
def tap_split(pk, qg):
    if pk < NPACK - 1:
        return (5, 4)
    # tail: Vector saturates (taps+combines+B0 leftovers) while PE has
    # slack once mm1 is gone, so PE takes 6 of 9 taps everywhere
    return (6, 3)


KPE_MAX = 6

_CACHED = {}


def build_nc():
    nc = bacc.Bacc("TRN2", target_bir_lowering=False, debug=False)

    x_d = nc.dram_tensor("x", [NPACK, NLD, 128, KC4 * LR * W], F16,
                         kind="ExternalInput").ap()
    cw_d = nc.dram_tensor("cw", [128, KC4 * 128], F16, kind="ExternalInput").ap()
    fw_d = nc.dram_tensor("fw", [128, MC2 * 128], F16, kind="ExternalInput").ap()
    cb2_d = nc.dram_tensor("cb2", [128, 1], F32, kind="ExternalInput").ap()
    fba2_d = nc.dram_tensor("fba2", [128, MC2], F32, kind="ExternalInput").ap()
    id_d = nc.dram_tensor("ident", [128, 128], F16, kind="ExternalInput").ap()
    y_d = nc.dram_tensor("y", [NPACK, NQG, 128, 2 * MC2 * NTV], F16,
                         kind="ExternalOutput").ap()

    with tile.TileContext(nc) as tc:
        build_body(nc, tc, x_d, cw_d, fw_d, cb2_d, fba2_d, id_d, y_d)
    nc.compile()
    return nc


def build_body(nc, tc, x_d, cw_d, fw_d, cb2_d, fba2_d, id_d, y_d):
    ctxs = []

    def pool(**kw):
        p = tc.tile_pool(**kw)
        ctxs.append(p)
        return p.__enter__()

    consts = pool(name="consts", bufs=1)
    xpool = pool(name="xs", bufs=3)
    fpads = pool(name="fpads", bufs=1)
    accp = pool(name="accp", bufs=4)
    tmpp = pool(name="tmpp", bufs=3)
    opool = pool(name="osb", bufs=3)
    ypool = pool(name="ysb", bufs=4)
    small = pool(name="small", bufs=1)
    diagp = pool(name="diagp", bufs=1)
    psA = pool(name="psA", bufs=2, space="PSUM")
    psD = pool(name="psD", bufs=2, space="PSUM")
    psY = pool(name="psY", bufs=2, space="PSUM")

    # ---- constants: FIRST on the sync ring, before the x strips. On a
    # busy ring the queues round-robin descriptor-by-descriptor, so a
    # const issued behind the strips would trickle in ~30us late. ----
    cw = consts.tile([128, KC4 * 128], F16)    # block-diag conv_w^T chunks
    nc.sync.dma_start(cw[:], cw_d)
    fw = consts.tile([128, MC2 * 128], F16)    # fuse_w^T dup'd on both halves
    nc.sync.dma_start(fw[:], fw_d)
    cb2 = consts.tile([128, 1], F32)
    nc.sync.dma_start(cb2[:], cb2_d)
    fba2 = consts.tile([128, MC2], F32)
    nc.sync.dma_start(fba2[:], fba2_d)
    ident = consts.tile([128, 128], F16)
    nc.sync.dma_start(ident[:], id_d)

    fpad = [fpads.tile([128, FPAD], F16, tag=f"fpad{pk}", name=f"fpad{pk}")
            for pk in range(NPACK)]
    for pk in range(NPACK):
        # halo-only zeroing: top row + row0 left halo, bottom row + slack,
        # and the interleaved right|left halo column pairs
        nc.gpsimd.memset(fpad[pk][:, 0:WP + 1], 0.0)
        nc.gpsimd.memset(fpad[pk][:, (H + 1) * WP:FPAD], 0.0)
        edge = fpad[pk][:, 2 * WP - 1:2 * WP - 1 + H * WP].rearrange(
            "p (r w) -> p r w", w=WP)[:, :, 0:2]
        nc.gpsimd.memset(edge, 0.0)

    xparts = [small.tile([128, NCH * 3], F32, tag=f"xp{pk}", name=f"xp{pk}")
              for pk in range(NPACK)]
    gsc = [small.tile([128, 9], F32, tag=f"g{pk}", name=f"g{pk}")
           for pk in range(NPACK)]
    diag9 = [diagp.tile([128, KPE_MAX * 128], F16, tag=f"d{pk}",
                        name=f"diag9{pk}") for pk in range(NPACK)]

    def tap_window(pk, t, r0, nrows):
        """fpad window for tap t over output rows [r0, r0+nrows), compact W
        cols per row (row stride WP)."""
        dy, dx = t // 3 - 1, t % 3 - 1
        base = (r0 + 1 + dy) * WP + 1 + dx
        return fpad[pk][:, base:base + nrows * WP].rearrange(
            "p (r w) -> p r w", w=WP)[:, :, 0:W]

    def phaseA_strip(pk, ld):
        """Generator: emits the strip load, then yields after each of the 8
        mm1 chunks so phase-A work can be woven finely between phase-B
        steps (coarse interleave lets A(1) pooling queue behind all of
        B(0)'s vector taps, stalling mm1 on psA recycle). x free dim is
        row-major (r, cc, w) so the strip loads in two halves and mm1
        starts on the first half."""
        r0 = ld * LR
        xt = xpool.tile([128, KC4 * LR * W], F16, tag="xt", name="xt")
        xtv = xt[:].rearrange("p (r cc w) -> p r cc w", r=LR, cc=KC4)
        half = KC4 * LR * W // 2
        if pk == 0 and ld == 0:
            # very first data: two quarter loads so mm1 starts sooner
            q4 = half // 2
            for h in range(2):
                nc.sync.dma_start(xt[:, h * q4:(h + 1) * q4],
                                  x_d[pk, ld][:, h * q4:(h + 1) * q4])
        else:
            nc.sync.dma_start(xt[:, 0:half], x_d[pk, ld][:, 0:half])
        nc.sync.dma_start(xt[:, half:2 * half],
                          x_d[pk, ld][:, half:2 * half])
        for j in range(LR // RS):
            ch = ld * (LR // RS) + j
            rr = r0 + j * RS
            pA = psA.tile([128, NT], F32, tag="pA", name="pA")
            for kc in range(KC4):
                nc.tensor.matmul(
                    pA[:],
                    cw[:, kc * 128:(kc + 1) * 128],
                    xtv[:, j * RS:(j + 1) * RS, kc:kc + 1, :],
                    start=(kc == 0), stop=(kc == KC4 - 1),
                )
            # f evict: relu(psum + conv_b) -> fpad fp16, 98-strided rows
            base = (rr + 1) * WP + 1
            dst = fpad[pk][:, base:base + RS * WP].rearrange(
                "p (r w) -> p r w", w=WP)[:, :, 0:W]
            nc.scalar.activation(dst, pA[:], AF.Relu, bias=cb2[:])
            # pooling partial sums (pre-relu, pre-bias)
            pv = pA[:].rearrange("p (r cb w) -> p cb r w", r=RS, cb=3, w=BR)
            nc.vector.tensor_reduce(
                xparts[pk][:, ch * 3:(ch + 1) * 3], pv, axis=AX.XY, op=ALU.add)
            yield

    def phaseA_final(pk):
        kpe = max(tap_split(pk, qg)[0] for qg in range(NQG))
        # dynamic kernels g; diag fp16 weight tiles only for the PE taps
        xp9 = small.tile([128, 9], F32, tag=f"xp9{pk}", name=f"xp9{pk}")
        nc.vector.tensor_reduce(
            xp9[:],
            xparts[pk][:].rearrange("p (br s cb) -> p br cb s",
                                    br=3, s=NCH // 3, cb=3),
            axis=AX.X, op=ALU.add)
        nc.vector.tensor_scalar(
            out=gsc[pk][:], in0=xp9[:], scalar1=1.0 / (BR * BR),
            scalar2=cb2[:], op0=ALU.mult, op1=ALU.add)
        for ti in range(kpe):
            nc.vector.tensor_scalar_mul(
                diag9[pk][:, ti * 128:(ti + 1) * 128], ident[:],
                gsc[pk][:, ti:ti + 1])

    def tap_pass(pk, qg, acc):
        """Generator: the group's Vector tap chain into acc
        (tensor_scalar_mul runs in 4x DVE mode, tensor_tensor add in 2x)."""
        kpe, kv = tap_split(pk, qg)
        r0 = qg * GRV
        for vi in range(kv):
            t = kpe + vi
            win = tap_window(pk, t, r0, GRV)
            if vi == 0:
                nc.vector.tensor_scalar_mul(acc[:], win, gsc[pk][:, t:t + 1])
            else:
                tmp = tmpp.tile([128, NTV], F16, tag="tmp", name="tmp")
                nc.vector.tensor_scalar_mul(tmp[:], win, gsc[pk][:, t:t + 1])
                nc.vector.tensor_tensor(acc[:], tmp[:], acc[:], ALU.add)
            yield

    def phaseB_front(pk, qg, osb, pre_acc=None):
        """Generator: vector taps + PE dw taps + combine for one 16-row
        group, writing the dw output into the caller-provided osb. If the
        tap chain was pre-emitted (pack boundary), pre_acc carries it."""
        kpe, kv = tap_split(pk, qg)
        r0 = qg * GRV
        if pre_acc is not None:
            acc = pre_acc
        else:
            acc = accp.tile([128, NTV], F16, tag="acc", name="acc")
            yield from tap_pass(pk, qg, acc)
        # PE taps + combine per chunk
        for q in range(RS):
            rr = r0 + q * RS
            p_start = (rr + 1) * WP + 1
            pD = psD.tile([128, NDW], F32, tag="pD", name="pD")
            for ti in range(kpe):
                dy, dx = ti // 3 - 1, ti % 3 - 1
                off = p_start + dy * WP + dx
                nc.tensor.matmul(
                    pD[:], diag9[pk][:, ti * 128:(ti + 1) * 128],
                    fpad[pk][:, off:off + NDW],
                    start=(ti == 0), stop=(ti == kpe - 1),
                )
            src = pD[:].rearrange("p (r w) -> p r w", w=WP)[:, :, 0:W]
            nc.vector.scalar_tensor_tensor(
                osb[:, q * NT:(q + 1) * NT], src, 1.0,
                acc[:, q * NT:(q + 1) * NT], ALU.mult, ALU.add)
            yield

    def phaseB_back(pk, qg, osb, v_evict=False):
        """Generator: mm2 512-col windows + bias evicts + store for one
        group. Emitted one group BEHIND the front so PE's in-order queue
        never stalls on the psY/evict ping-pong (dw of the next group runs
        while Scalar drains evicts). v_evict alternates evicts onto Vector
        for the final groups, where Vector is otherwise idle and Scalar
        alone paces the drain."""
        ysb = ypool.tile([128, 2 * MC2 * NTV], F16, tag="ysb", name="ysb")
        ysbv = ysb[:].rearrange("p (s mc f) -> p s mc f", s=2, mc=MC2)
        for mc in range(MC2):
            for w3 in range(NW3):
                pY = psY.tile([128, 2 * MMW], F32, tag="pY", name="pY")
                nc.tensor.matmul(
                    pY[:, 0:MMW], fw[0:64, mc * 128:(mc + 1) * 128],
                    osb[0:64, w3 * MMW:(w3 + 1) * MMW],
                    start=True, stop=True)
                nc.tensor.matmul(
                    pY[:, MMW:2 * MMW], fw[64:128, mc * 128:(mc + 1) * 128],
                    osb[64:128, w3 * MMW:(w3 + 1) * MMW],
                    start=True, stop=True)
                ysrc = pY[:].rearrange("p (s f) -> p s f", s=2)
                dst = ysbv[:, :, mc:mc + 1,
                           w3 * MMW:(w3 + 1) * MMW].rearrange(
                    "p s mc f -> p (s mc) f")
                if v_evict and (mc * NW3 + w3) % 2 == 1:
                    nc.vector.tensor_scalar_add(dst, ysrc,
                                                fba2[:, mc:mc + 1])
                else:
                    nc.scalar.activation(dst, ysrc, AF.Identity,
                                         bias=fba2[:, mc:mc + 1])
                yield
        nc.scalar.dma_start(y_d[pk, qg], ysb[:])

    END = object()

    def drain(gen):
        for _ in gen:
            pass

    # Software pipeline:
    #   A(0); then per 16-row group: front(g) [V taps + PE dw + combine]
    #   woven with back(g-1) [mm2 + evicts + store, one group BEHIND so
    #   PE's in-order queue never stalls on the psY/evict ping-pong] and,
    #   during pack 0's groups, with A(1) chunks [fine weave keeps A(1)
    #   pooling close behind its mm1 so psA recycle never stalls].
    for ld in range(NLD):
        drain(phaseA_strip(0, ld))
    phaseA_final(0)

    def a_iter(pk):
        for ld in range(NLD):
            yield from phaseA_strip(pk, ld)
    anext = a_iter(1) if NPACK > 1 else None
    prev_back = None
    pre_accs = {}
    bi = 0
    for pk in range(NPACK):
        for qg in range(NQG):
            tail = pk == NPACK - 1
            kv = tap_split(pk, qg)[1]
            osb = opool.tile([128, NTV], F16, tag="osb", name="osb")
            own_back = None
            fi = 0
            for _ in phaseB_front(pk, qg, osb, pre_acc=pre_accs.pop(
                    (pk, qg), None)):
                bi += 1
                fi += 1
                if prev_back is not None and next(prev_back, END) is END:
                    prev_back = None
                if anext is not None and pk == 0 and bi % 2 == 0:
                    if next(anext, END) is END:
                        anext = None
                        # emit final(1) NOW so it sits mid-B(0) in the
                        # Vector queue instead of delaying B(1)'s taps
                        phaseA_final(pk + 1)
                # tail groups can't fully hide mm2/evicts behind a next
                # group's dw (the pack ends), so start each back mid-front
                # (mm2 window w only needs combines up to chunk w+1) and
                # let leftovers weave into the next front
                if tail:
                    if own_back is None and fi >= kv + 2:
                        own_back = phaseB_back(
                            pk, qg, osb, v_evict=(qg >= NQG - 3))
                    elif own_back is not None:
                        next(own_back, END)
            if tail:
                if prev_back is not None:
                    drain(prev_back)
                prev_back = own_back
            else:
                prev_back = phaseB_back(pk, qg, osb)
        if pk + 1 < NPACK and anext is not None:
            drain(anext)
            anext = None
            phaseA_final(pk + 1)
    if prev_back is not None:
        drain(prev_back)

    for p in reversed(ctxs):
        p.__exit__(None, None, None)


def _prep(inputs):
    x = np.asarray(inputs["x"], dtype=np.float32)
    conv_w = np.asarray(inputs["conv_w"], dtype=np.float32)
    conv_b = np.asarray(inputs["conv_b"], dtype=np.float32)
    dw_b = np.asarray(inputs["dw_b"], dtype=np.float32)
    fuse_w = np.asarray(inputs["fuse_w"], dtype=np.float32)
    fuse_b = np.asarray(inputs["fuse_b"], dtype=np.float32)

    cwT = np.ascontiguousarray(conv_w.T)                      # [256, 64]
    cw = np.zeros((128, KC4 * 128), np.float16)               # block-diag
    for kc in range(KC4):
        blk = cwT[kc * 64:(kc + 1) * 64, :]                   # [64 k, 64 m]
        cw[0:64, kc * 128:kc * 128 + 64] = blk
        cw[64:128, kc * 128 + 64:(kc + 1) * 128] = blk
    fwT = np.ascontiguousarray(fuse_w.T)                      # [64, 256]
    fw = np.zeros((128, MC2 * 128), np.float16)
    for mc in range(MC2):
        blk = fwT[:, mc * 128:(mc + 1) * 128]
        fw[0:64, mc * 128:(mc + 1) * 128] = blk
        fw[64:128, mc * 128:(mc + 1) * 128] = blk
    cb2 = np.tile(conv_b, 2)[:, None].astype(np.float32)      # [128, 1]
    fba_flat = (fuse_b + fuse_w @ dw_b).astype(np.float32)    # [256]
    fba2 = np.stack([fba_flat[mc * 128:(mc + 1) * 128]
                     for mc in range(MC2)], axis=1)           # [128, 2]
    ident = np.eye(128, dtype=np.float16)

    # pre-cast x to fp16 on the host (the device matmuls consume fp16
    # anyway; halves HBM read traffic) and tile it row-major so every load
    # DMA descriptor is one 12.3KB half-strip partition line:
    # xh[core, pk, ld, si*64+cl, (r, cc, w)] = x[core*4+2pk+si, cc*64+cl,
    #                                            ld*LR+r, w]
    xh = x.reshape(NCORES, NPACK, 2, KC4, 64, NLD, LR, W).astype(np.float16)
    xh = np.ascontiguousarray(xh.transpose(0, 1, 5, 2, 4, 6, 3, 7)).reshape(
        NCORES, NPACK, NLD, 128, KC4 * LR * W)
    in_maps = []
    for i in range(NCORES):
        in_maps.append({
            "x": xh[i],
            "cw": cw,
            "fw": fw,
            "cb2": cb2,
            "fba2": fba2,
            "ident": ident,
        })
    return in_maps


def run(inputs, trace=False):
    if "nc" not in _CACHED:
        _CACHED["nc"] = build_nc()
    nc = _CACHED["nc"]
    in_maps = _prep(inputs)
    res = run_bass_kernel_spmd(nc, in_maps, list(range(NCORES)), trace=trace)
    # yh[pk, qg, c, s, mc, f] -> y[core*4+2pk+s, mc*128+c, qg*GRV*W+f]
    yh = np.stack([res.results[i]["y"] for i in range(NCORES)], axis=0)
    yh = yh.reshape(NCORES, NPACK, NQG, 128, 2, MC2, NTV).astype(np.float32)
    y = yh.transpose(0, 1, 4, 5, 3, 2, 6).reshape(N, P, H, W)
    return y, res


def kernel(**inputs):
    y, _ = run(inputs, trace=False)
    return y


# revision 57
# speedup vs baseline: 1.0079x; 1.0079x over previous
"""Trainium2 Bass kernel for DCMLayer: 1x1 conv -> per-sample dynamic 3x3
depthwise conv -> 1x1 fuse conv, data-parallel over 8 NeuronCores.

Contract: kernel(**inputs) takes the FULL unsharded inputs
(x[32,256,96,96], conv_w[64,256], conv_b[64], dw_b[64], fuse_w[256,64],
fuse_b[256]) and returns the full y[32,256,96,96] float32.

v5 (over v2's 202us baseline; ~176-179us fast-regime, the shared HW shows
~15% run-to-run throttling variance):
- x pre-cast to fp16 on host (the matmuls consume fp16 anyway; halves
  HBM reads) and host-tiled row-major so each half-strip load is one
  [128, 6144] DMA with 12.3KB contiguous descriptor lines (v2's 6KB
  lines were descriptor-rate-bound at ~60% HBM) and mm1 starts on the
  first half-strip.
- consts load FIRST on the sync ring: DMA queues round-robin
  descriptor-by-descriptor, so consts issued behind the strips trickle
  in ~30-50us late and gate the whole B phase.
- dw taps split PE (diagonal fp16 matmuls into PSUM) / Vector (4x-mode
  tensor_scalar_mul + 2x tensor_tensor add over 16-row groups),
  (5,4) mid-pack and (6,3) in the tail pack (Vector saturates there,
  PE has slack once mm1 is gone). GpSimd supports neither
  TensorScalarPtr nor PSUM access; fp8 dw blows the 2e-2 tolerance
  (measured 3e-2).
- per-chunk combine osb = pD + acc on Vector; mm2 as 512-col matmuls
  (PSUM bank max) over the group osb; samples a/b land in one 2-bank
  PSUM tile (b at col 512) -> single strided Scalar bias-evict per
  (mc, window); y stored per 16-row group ([128, 6144], 12.3KB lines),
  host untangles the layout.
- generator-woven software pipeline: pack-1 phase-A chunks interleave
  every other phase-B step (keeps pooling close behind mm1 so psA
  recycle never stalls); each group's mm2/evicts emit one group BEHIND
  its dw (PE's in-order queue never stalls on the psY/evict ping-pong);
  every tail group's back-phase starts mid-front (hybrid weave) and
  the last three groups' evicts alternate Scalar/Vector to share the
  end drain.
"""
import numpy as np

import concourse.bacc as bacc
import concourse.bass as bass
import concourse.tile as tile
from concourse import mybir
from concourse.bass_utils import run_bass_kernel_spmd

F32 = mybir.dt.float32
F16 = mybir.dt.float16
AF = mybir.ActivationFunctionType
ALU = mybir.AluOpType
AX = mybir.AxisListType

# Problem geometry (hardcoded per contract)
N, C, H, W = 32, 256, 96, 96
Cm, P = 64, 256
HW = H * W           # 9216
NCORES = 8
NLOC = N // NCORES   # 4 samples per core
NPACK = NLOC // 2    # 2 two-sample packs per core
KC4 = C // 64        # 4 K=64(x2 samples) contraction chunks for mm1
MC2 = P // 128       # 2 M=128 output chunks for mm2

WP = W + 2           # padded row width 98
FPAD = WP * (H + 2) + 2  # padded f buffer 9606 (+2 slack for corner taps)
RS = 4               # rows per compute chunk
NCH = H // RS        # 24 chunks per pack
NT = RS * W          # 384 = compute tile free size
NDW = RS * WP        # 392 dw output positions per chunk
GRV = 16             # rows per group (vector taps, osb, y store)
NTV = GRV * W        # 1536 = group free size
NQG = H // GRV       # 6 groups per pack
MMW = 512            # mm2 moving-window columns (PSUM bank max)
NW3 = NTV // MMW     # 3 mm2 windows per group
# uneven x load strips (r0, nrows): a short LAST strip shrinks the
# trailing mm1 after the final load arrives (head is load-paced)
STRIPS = [(0, 40), (40, 40), (80, 16)]
NLD = len(STRIPS)
LRMAX = max(nr for _, nr in STRIPS)
BR = 32              # pooling block rows/cols

# dw tap split per (pack, qgroup): (PE diag-matmul taps, Vector taps).
# GpSimd/Pool supports neither TensorScalarPtr nor PSUM access, so taps go
# PE/DVE only. Alternating per group balances the two engines; the tail
# pack (no concurrent mm1) shifts one tap toward PE.
def tap_split(pk, qg):
    if pk < NPACK - 1:
        return (5, 4)
    # tail: Vector saturates (taps+combines+B0 leftovers) while PE has
    # slack once mm1 is gone, so PE takes 6 of 9 taps everywhere
    return (6, 3)


KPE_MAX = 6

_CACHED = {}


def build_nc():
    nc = bacc.Bacc("TRN2", target_bir_lowering=False, debug=False)

    x_d = nc.dram_tensor("x", [NPACK, 128, H * KC4 * W], F16,
                         kind="ExternalInput").ap()
    cw_d = nc.dram_tensor("cw", [128, KC4 * 128], F16, kind="ExternalInput").ap()
    fw_d = nc.dram_tensor("fw", [128, MC2 * 128], F16, kind="ExternalInput").ap()
    cb2_d = nc.dram_tensor("cb2", [128, 1], F32, kind="ExternalInput").ap()
    fba2_d = nc.dram_tensor("fba2", [128, MC2], F32, kind="ExternalInput").ap()
    id_d = nc.dram_tensor("ident", [128, 128], F16, kind="ExternalInput").ap()
    y_d = nc.dram_tensor("y", [NPACK, NQG, 128, 2 * MC2 * NTV], F16,
                         kind="ExternalOutput").ap()

    with tile.TileContext(nc) as tc:
        build_body(nc, tc, x_d, cw_d, fw_d, cb2_d, fba2_d, id_d, y_d)
    nc.compile()
    return nc


def build_body(nc, tc, x_d, cw_d, fw_d, cb2_d, fba2_d, id_d, y_d):
    ctxs = []

    def pool(**kw):
        p = tc.tile_pool(**kw)
        ctxs.append(p)
        return p.__enter__()

    consts = pool(name="consts", bufs=1)
    xpool = pool(name="xs", bufs=3)
    fpads = pool(name="fpads", bufs=1)
    accp = pool(name="accp", bufs=4)
    tmpp = pool(name="tmpp", bufs=3)
    opool = pool(name="osb", bufs=3)
    ypool = pool(name="ysb", bufs=3)
    small = pool(name="small", bufs=1)
    diagp = pool(name="diagp", bufs=1)
    psA = pool(name="psA", bufs=2, space="PSUM")
    psD = pool(name="psD", bufs=2, space="PSUM")
    psY = pool(name="psY", bufs=2, space="PSUM")

    # ---- constants: FIRST on the sync ring, before the x strips. On a
    # busy ring the queues round-robin descriptor-by-descriptor, so a
    # const issued behind the strips would trickle in ~30us late. ----
    cw = consts.tile([128, KC4 * 128], F16)    # block-diag conv_w^T chunks
    nc.sync.dma_start(cw[:], cw_d)
    fw = consts.tile([128, MC2 * 128], F16)    # fuse_w^T dup'd on both halves
    nc.sync.dma_start(fw[:], fw_d)
    cb2 = consts.tile([128, 1], F32)
    nc.sync.dma_start(cb2[:], cb2_d)
    fba2 = consts.tile([128, MC2], F32)
    nc.sync.dma_start(fba2[:], fba2_d)
    ident = consts.tile([128, 128], F16)
    nc.sync.dma_start(ident[:], id_d)

    fpad = [fpads.tile([128, FPAD], F16, tag=f"fpad{pk}", name=f"fpad{pk}")
            for pk in range(NPACK)]
    for pk in range(NPACK):
        # halo-only zeroing: top row + row0 left halo, bottom row + slack,
        # and the interleaved right|left halo column pairs
        nc.gpsimd.memset(fpad[pk][:, 0:WP + 1], 0.0)
        nc.gpsimd.memset(fpad[pk][:, (H + 1) * WP:FPAD], 0.0)
        edge = fpad[pk][:, 2 * WP - 1:2 * WP - 1 + H * WP].rearrange(
            "p (r w) -> p r w", w=WP)[:, :, 0:2]
        nc.gpsimd.memset(edge, 0.0)

    xparts = [small.tile([128, NCH * 3], F32, tag=f"xp{pk}", name=f"xp{pk}")
              for pk in range(NPACK)]
    gsc = [small.tile([128, 9], F32, tag=f"g{pk}", name=f"g{pk}")
           for pk in range(NPACK)]
    diag9 = [diagp.tile([128, KPE_MAX * 128], F16, tag=f"d{pk}",
                        name=f"diag9{pk}") for pk in range(NPACK)]

    def tap_window(pk, t, r0, nrows):
        """fpad window for tap t over output rows [r0, r0+nrows), compact W
        cols per row (row stride WP)."""
        dy, dx = t // 3 - 1, t % 3 - 1
        base = (r0 + 1 + dy) * WP + 1 + dx
        return fpad[pk][:, base:base + nrows * WP].rearrange(
            "p (r w) -> p r w", w=WP)[:, :, 0:W]

    def phaseA_strip(pk, ld):
        """Generator: emits the strip load, then yields after each mm1
        chunk so phase-A work can be woven finely between phase-B steps
        (coarse interleave lets A(1) pooling queue behind all of B(0)'s
        vector taps, stalling mm1 on psA recycle). x free dim is row-major
        (r, cc, w) so strips load in halves (quarters for the very first
        data) and mm1 starts on the first piece."""
        r0, nr = STRIPS[ld]
        row = KC4 * W
        xt = xpool.tile([128, LRMAX * row], F16, tag="xt", name="xt")
        xtv = xt[:].rearrange("p (r cc w) -> p r cc w", r=LRMAX, cc=KC4)
        xsrc = x_d[pk][:, r0 * row:(r0 + nr) * row]
        npiece = 4 if (pk == 0 and ld == 0) else 2
        piece = nr * row // npiece
        for h in range(npiece):
            nc.sync.dma_start(xt[:, h * piece:(h + 1) * piece],
                              xsrc[:, h * piece:(h + 1) * piece])
        for j in range(nr // RS):
            ch = r0 // RS + j
            rr = r0 + j * RS
            pA = psA.tile([128, NT], F32, tag="pA", name="pA")
            for kc in range(KC4):
                nc.tensor.matmul(
                    pA[:],
                    cw[:, kc * 128:(kc + 1) * 128],
                    xtv[:, j * RS:(j + 1) * RS, kc:kc + 1, :],
                    start=(kc == 0), stop=(kc == KC4 - 1),
                )
            # f evict: relu(psum + conv_b) -> fpad fp16, 98-strided rows
            base = (rr + 1) * WP + 1
            dst = fpad[pk][:, base:base + RS * WP].rearrange(
                "p (r w) -> p r w", w=WP)[:, :, 0:W]
            nc.scalar.activation(dst, pA[:], AF.Relu, bias=cb2[:])
            # pooling partial sums (pre-relu, pre-bias)
            pv = pA[:].rearrange("p (r cb w) -> p cb r w", r=RS, cb=3, w=BR)
            nc.vector.tensor_reduce(
                xparts[pk][:, ch * 3:(ch + 1) * 3], pv, axis=AX.XY, op=ALU.add)
            yield

    def phaseA_final(pk):
        kpe = max(tap_split(pk, qg)[0] for qg in range(NQG))
        # dynamic kernels g; diag fp16 weight tiles only for the PE taps
        xp9 = small.tile([128, 9], F32, tag=f"xp9{pk}", name=f"xp9{pk}")
        nc.vector.tensor_reduce(
            xp9[:],
            xparts[pk][:].rearrange("p (br s cb) -> p br cb s",
                                    br=3, s=NCH // 3, cb=3),
            axis=AX.X, op=ALU.add)
        nc.vector.tensor_scalar(
            out=gsc[pk][:], in0=xp9[:], scalar1=1.0 / (BR * BR),
            scalar2=cb2[:], op0=ALU.mult, op1=ALU.add)
        for ti in range(kpe):
            nc.vector.tensor_scalar_mul(
                diag9[pk][:, ti * 128:(ti + 1) * 128], ident[:],
                gsc[pk][:, ti:ti + 1])

    def tap_pass(pk, qg, acc):
        """Generator: the group's Vector tap chain into acc
        (tensor_scalar_mul runs in 4x DVE mode, tensor_tensor add in 2x)."""
        kpe, kv = tap_split(pk, qg)
        r0 = qg * GRV
        for vi in range(kv):
            t = kpe + vi
            win = tap_window(pk, t, r0, GRV)
            if vi == 0:
                nc.vector.tensor_scalar_mul(acc[:], win, gsc[pk][:, t:t + 1])
            else:
                tmp = tmpp.tile([128, NTV], F16, tag="tmp", name="tmp")
                nc.vector.tensor_scalar_mul(tmp[:], win, gsc[pk][:, t:t + 1])
                nc.vector.tensor_tensor(acc[:], tmp[:], acc[:], ALU.add)
            yield

    def phaseB_front(pk, qg, osb, pre_acc=None):
        """Generator: vector taps + PE dw taps + combine for one 16-row
        group, writing the dw output into the caller-provided osb. If the
        tap chain was pre-emitted (pack boundary), pre_acc carries it."""
        kpe, kv = tap_split(pk, qg)
        r0 = qg * GRV
        if pre_acc is not None:
            acc = pre_acc
        else:
            acc = accp.tile([128, NTV], F16, tag="acc", name="acc")
            yield from tap_pass(pk, qg, acc)
        # PE taps + combine per chunk
        for q in range(RS):
            rr = r0 + q * RS
            p_start = (rr + 1) * WP + 1
            pD = psD.tile([128, NDW], F32, tag="pD", name="pD")
            for ti in range(kpe):
                dy, dx = ti // 3 - 1, ti % 3 - 1
                off = p_start + dy * WP + dx
                nc.tensor.matmul(
                    pD[:], diag9[pk][:, ti * 128:(ti + 1) * 128],
                    fpad[pk][:, off:off + NDW],
                    start=(ti == 0), stop=(ti == kpe - 1),
                )
            src = pD[:].rearrange("p (r w) -> p r w", w=WP)[:, :, 0:W]
            nc.vector.scalar_tensor_tensor(
                osb[:, q * NT:(q + 1) * NT], src, 1.0,
                acc[:, q * NT:(q + 1) * NT], ALU.mult, ALU.add)
            yield

    def phaseB_back(pk, qg, osb, v_evict=False):
        """Generator: mm2 512-col windows + bias evicts + store for one
        group. Emitted one group BEHIND the front so PE's in-order queue
        never stalls on the psY/evict ping-pong (dw of the next group runs
        while Scalar drains evicts). v_evict alternates evicts onto Vector
        for the final groups, where Vector is otherwise idle and Scalar
        alone paces the drain."""
        ysb = ypool.tile([128, 2 * MC2 * NTV], F16, tag="ysb", name="ysb")
        ysbv = ysb[:].rearrange("p (s mc f) -> p s mc f", s=2, mc=MC2)
        for mc in range(MC2):
            for w3 in range(NW3):
                pY = psY.tile([128, 2 * MMW], F32, tag="pY", name="pY")
                nc.tensor.matmul(
                    pY[:, 0:MMW], fw[0:64, mc * 128:(mc + 1) * 128],
                    osb[0:64, w3 * MMW:(w3 + 1) * MMW],
                    start=True, stop=True)
                nc.tensor.matmul(
                    pY[:, MMW:2 * MMW], fw[64:128, mc * 128:(mc + 1) * 128],
                    osb[64:128, w3 * MMW:(w3 + 1) * MMW],
                    start=True, stop=True)
                ysrc = pY[:].rearrange("p (s f) -> p s f", s=2)
                dst = ysbv[:, :, mc:mc + 1,
                           w3 * MMW:(w3 + 1) * MMW].rearrange(
                    "p s mc f -> p (s mc) f")
                if v_evict and (mc * NW3 + w3) % 2 == 1:
                    nc.vector.tensor_scalar_add(dst, ysrc,
                                                fba2[:, mc:mc + 1])
                else:
                    nc.scalar.activation(dst, ysrc, AF.Identity,
                                         bias=fba2[:, mc:mc + 1])
                yield
        nc.scalar.dma_start(y_d[pk, qg], ysb[:])

    END = object()

    def drain(gen):
        for _ in gen:
            pass

    # Software pipeline:
    #   A(0); then per 16-row group: front(g) [V taps + PE dw + combine]
    #   woven with back(g-1) [mm2 + evicts + store, one group BEHIND so
    #   PE's in-order queue never stalls on the psY/evict ping-pong] and,
    #   during pack 0's groups, with A(1) chunks [fine weave keeps A(1)
    #   pooling close behind its mm1 so psA recycle never stalls].
    for ld in range(NLD):
        drain(phaseA_strip(0, ld))
    phaseA_final(0)

    def a_iter(pk):
        for ld in range(NLD):
            yield from phaseA_strip(pk, ld)
    anext = a_iter(1) if NPACK > 1 else None
    prev_back = None
    pre_accs = {}
    bi = 0
    for pk in range(NPACK):
        for qg in range(NQG):
            tail = pk == NPACK - 1
            kv = tap_split(pk, qg)[1]
            osb = opool.tile([128, NTV], F16, tag="osb", name="osb")
            own_back = None
            fi = 0
            for _ in phaseB_front(pk, qg, osb, pre_acc=pre_accs.pop(
                    (pk, qg), None)):
                bi += 1
                fi += 1
                if prev_back is not None and next(prev_back, END) is END:
                    prev_back = None
                if anext is not None and pk == 0 and bi % 2 == 0:
                    if next(anext, END) is END:
                        anext = None
                        # emit final(1) NOW so it sits mid-B(0) in the
                        # Vector queue instead of delaying B(1)'s taps
                        phaseA_final(pk + 1)
                # tail groups can't fully hide mm2/evicts behind a next
                # group's dw (the pack ends), so start each back mid-front
                # (mm2 window w only needs combines up to chunk w+1) and
                # let leftovers weave into the next front
                if tail:
                    if own_back is None and fi >= kv + 2:
                        own_back = phaseB_back(
                            pk, qg, osb, v_evict=(qg >= NQG - 3))
                    elif own_back is not None:
                        next(own_back, END)
            if tail:
                if prev_back is not None:
                    drain(prev_back)
                prev_back = own_back
            else:
                prev_back = phaseB_back(pk, qg, osb)
        if pk + 1 < NPACK and anext is not None:
            drain(anext)
            anext = None
            phaseA_final(pk + 1)
    if prev_back is not None:
        drain(prev_back)

    for p in reversed(ctxs):
        p.__exit__(None, None, None)


def _prep(inputs):
    x = np.asarray(inputs["x"], dtype=np.float32)
    conv_w = np.asarray(inputs["conv_w"], dtype=np.float32)
    conv_b = np.asarray(inputs["conv_b"], dtype=np.float32)
    dw_b = np.asarray(inputs["dw_b"], dtype=np.float32)
    fuse_w = np.asarray(inputs["fuse_w"], dtype=np.float32)
    fuse_b = np.asarray(inputs["fuse_b"], dtype=np.float32)

    cwT = np.ascontiguousarray(conv_w.T)                      # [256, 64]
    cw = np.zeros((128, KC4 * 128), np.float16)               # block-diag
    for kc in range(KC4):
        blk = cwT[kc * 64:(kc + 1) * 64, :]                   # [64 k, 64 m]
        cw[0:64, kc * 128:kc * 128 + 64] = blk
        cw[64:128, kc * 128 + 64:(kc + 1) * 128] = blk
    fwT = np.ascontiguousarray(fuse_w.T)                      # [64, 256]
    fw = np.zeros((128, MC2 * 128), np.float16)
    for mc in range(MC2):
        blk = fwT[:, mc * 128:(mc + 1) * 128]
        fw[0:64, mc * 128:(mc + 1) * 128] = blk
        fw[64:128, mc * 128:(mc + 1) * 128] = blk
    cb2 = np.tile(conv_b, 2)[:, None].astype(np.float32)      # [128, 1]
    fba_flat = (fuse_b + fuse_w @ dw_b).astype(np.float32)    # [256]
    fba2 = np.stack([fba_flat[mc * 128:(mc + 1) * 128]
                     for mc in range(MC2)], axis=1)           # [128, 2]
    ident = np.eye(128, dtype=np.float16)

    # pre-cast x to fp16 on the host (the device matmuls consume fp16
    # anyway; halves HBM read traffic) and tile it row-major so every load
    # DMA descriptor is one 12.3KB half-strip partition line:
    # xh[core, pk, ld, si*64+cl, (r, cc, w)] = x[core*4+2pk+si, cc*64+cl,
    #                                            ld*LR+r, w]
    xh = x.reshape(NCORES, NPACK, 2, KC4, 64, H, W).astype(np.float16)
    xh = np.ascontiguousarray(xh.transpose(0, 1, 2, 4, 5, 3, 6)).reshape(
        NCORES, NPACK, 128, H * KC4 * W)
    in_maps = []
    for i in range(NCORES):
        in_maps.append({
            "x": xh[i],
            "cw": cw,
            "fw": fw,
            "cb2": cb2,
            "fba2": fba2,
            "ident": ident,
        })
    return in_maps


def run(inputs, trace=False):
    if "nc" not in _CACHED:
        _CACHED["nc"] = build_nc()
    nc = _CACHED["nc"]
    in_maps = _prep(inputs)
    res = run_bass_kernel_spmd(nc, in_maps, list(range(NCORES)), trace=trace)
    # yh[pk, qg, c, s, mc, f] -> y[core*4+2pk+s, mc*128+c, qg*GRV*W+f]
    yh = np.stack([res.results[i]["y"] for i in range(NCORES)], axis=0)
    yh = yh.reshape(NCORES, NPACK, NQG, 128, 2, MC2, NTV).astype(np.float32)
    y = yh.transpose(0, 1, 4, 5, 3, 2, 6).reshape(N, P, H, W)
    return y, res


def kernel(**inputs):
    y, _ = run(inputs, trace=False)
    return y
